# revision 2
# baseline (speedup 1.0000x reference)
"""SSIM loss kernel v2 for Trainium2 (Bass/Tile), 8-core data parallel.

Math (per 512x512 plane, 11x11 gaussian window G, zero "same" padding):
  F1 = X+Y, F2 = X-Y, uF = F1^2/2, vF = F2^2/2
  s = G2(G1 F1), d = G2(G1 F2)        (separable blur via banded matmuls)
  U' = G2(G1 (uF+vF) + CC) = sg_x+sg_y + mu_x^2+mu_y^2 + CC  (interior)
  W' = G2(G1 (uF-vF) + CC) = 2 sg_xy + 2 mu_x mu_y + CC      (CC = C1+C2)
  u = s^2/2, v = d^2/2
  A1 = u-v+C1, B1 = u+v+C1, A2 = W'-A1, B2 = U'-B1
  ssim = (A1*A2)/(B1*B2);  loss = 1 - mean(ssim)

vs v1: bf16 matmuls (1 cyc/row vs 4), banded windows (542 vs 960 rows per
src-block), A2/B2 closed on the TensorEngine via negated-identity accumulate,
per-plane ssim summed in PSUM via identity accumulate, CC folded into pass1
copy bias, 1/Dt via Abs_reciprocal_sqrt, elementwise work spread across
DVE / Act / Pool (Pool is SBUF-only tensor_tensor/tensor_scalar/copy).
"""

import sys

for _p in ("/opt/trn_rl_repo",):
    if _p not in sys.path:
        sys.path.insert(0, _p)

import numpy as np
import ml_dtypes

import concourse.bass as bass
import concourse.bacc as bacc
import concourse.mybir as mybir
import concourse.tile as tile
from concourse.bass_utils import run_bass_kernel_spmd

F32 = mybir.dt.float32
BF16 = mybir.dt.bfloat16
AOP = mybir.AluOpType
AFT = mybir.ActivationFunctionType

N_CORES = 8
BATCH = 16
CH = 3
H = W = 512
PLANES = (BATCH // N_CORES) * CH  # 6 planes per core
WIN_SIZE = 11
SIGMA = 1.5
HALF = WIN_SIZE // 2
C1 = 0.01 ** 2
C2 = 0.03 ** 2
CC = C1 + C2
INVR2 = float(np.float32(1.0) / np.sqrt(np.float32(2.0)))

WIN = [(0, 133), (123, 138), (251, 138), (379, 133)]
OFF = [0, 133, 271, 409]
CATW = 542
CONSTW = 3 * CATW + 256  # 3 band copies + negI + posI

# default engine/strategy choices (overridable for the sim-driven search)
CFG = dict(
    tf1="act",      # TF1 drain: dve | act
    tf2="act",      # TF2 drain: dve | act
    tu="act",       # TU drain (+CC bias): act | dve
    tw="act",       # TW drain (+CC bias): act | dve
    a1b1="dve",     # A1/B1 combine: dve | pool
    f1="dve",       # F1: dve | pool
    f2="pool",      # F2: dve | pool
    rr="dve",       # RR = RSQ^2: pool | dve
    ssim_eng="dve", # final ssim product: dve | pool
    drain_w=1024,   # pass1 drain width: 512 | 1024 | 2048
    acc="amr",      # ssim accumulate: pe (psum identity) | pool (sbuf adds)
                    #                  | amr (DVE fused product+reduce)
    ps1_bufs=None,  # pass1 psum bufs (None = auto)
    ps2b_bufs=1,    # a2/b2 psum bufs
    dma_chunks=2,   # input DMA split for earlier pipeline start
)


def _gauss1d():
    coords = np.arange(WIN_SIZE, dtype=np.float32) - HALF
    g = np.exp(-(coords ** 2) / np.float32(2.0 * SIGMA ** 2)).astype(np.float32)
    g = g / g.sum(dtype=np.float32)
    return g.astype(np.float32)


def _band_np():
    g = _gauss1d()
    A = np.zeros((H, H), dtype=np.float32)
    for i in range(H):
        lo = max(0, i - HALF)
        hi = min(H, i + HALF + 1)
        for j in range(lo, hi):
            A[i, j] = g[j - i + HALF]
    segs = []
    for kt in range(4):
        ns, w = WIN[kt]
        segs.append(A[ns:ns + w, kt * 128:(kt + 1) * 128].T.copy())
    cat = np.concatenate(segs, axis=1)
    assert cat.shape == (128, CATW)
    return cat


def _consts_np():
    """[128, 3*CATW + 256] bf16: [BM | 0.5*BM | -0.5*BM | -I | I]."""
    cat = _band_np()
    eye = np.eye(128, dtype=np.float32)
    full = np.concatenate([cat, 0.5 * cat, -0.5 * cat, -eye, eye], axis=1)
    return full.astype(ml_dtypes.bfloat16)


def build_nc(planes=PLANES, cfg=None):
    C = dict(CFG)
    if cfg:
        C.update(cfg)

    ps1_bufs = C["ps1_bufs"]
    if ps1_bufs is None:
        ps1_bufs = 2 if C["drain_w"] <= 1024 else 1

    nc = bacc.Bacc(None)
    pred_d = nc.declare_dram_parameter("pred", [planes, H, W], F32, isOutput=False)
    targ_d = nc.declare_dram_parameter("target", [planes, H, W], F32, isOutput=False)
    cst_d = nc.declare_dram_parameter("consts", [128, CONSTW], BF16, isOutput=False)
    acc_d = nc.declare_dram_parameter("acc", [128, planes * 4], F32, isOutput=True)

    def tt(eng, out, a, b, op):
        getattr(nc, {"dve": "vector", "pool": "gpsimd"}[eng]).tensor_tensor(
            out, a, b, op)

    with tile.TileContext(nc) as tc:
        with (
            tc.tile_pool(name="const", bufs=1) as constp,
            tc.tile_pool(name="xy", bufs=2) as xyp,
            tc.tile_pool(name="fields", bufs=C.get("fld_bufs", 2)) as fldp,
            tc.tile_pool(name="transposed", bufs=2) as trp,
            tc.tile_pool(name="post", bufs=2) as pp,
            tc.tile_pool(name="accp", bufs=1) as accp,
            tc.tile_pool(name="ps1", bufs=ps1_bufs, space="PSUM") as ps1,
            tc.tile_pool(name="ps2a", bufs=1, space="PSUM") as ps2a,
            tc.tile_pool(name="ps2b", bufs=C["ps2b_bufs"], space="PSUM") as ps2b,
            tc.tile_pool(name="psacc", bufs=1, space="PSUM") as psaccp,
        ):
            CST = constp.tile([128, CONSTW], BF16)
            nc.sync.dma_start(CST[:], cst_d[:])
            negI = CST[:, 3 * CATW: 3 * CATW + 128]
            posI = CST[:, 3 * CATW + 128: 3 * CATW + 256]
            acc = accp.tile([128, planes * 4], F32)
            nc.vector.memset(acc[:], 0.0)
            scr_acc = accp.tile([128, 2], F32)
            nc.vector.memset(scr_acc[:], 0.0)

            def conv_pass(dst, srcs, blk, stop=True):
                mms = []
                for (T, seg) in srcs:
                    for kt in range(4):
                        lhsT = T[:, kt * 512 + blk * 128: kt * 512 + (blk + 1) * 128]
                        ns, w = WIN[kt]
                        off = seg * CATW + OFF[kt]
                        mms.append((dst[:, ns:ns + w], lhsT, CST[:, off:off + w]))
                n_mm = len(mms)
                for i, (o, l, r) in enumerate(mms):
                    nc.tensor.matmul(o, l, r, start=(i == 0),
                                     stop=(stop and i == n_mm - 1))

            def drain(eng, dst, src_ps, bias):
                if eng == "act":
                    nc.scalar.activation(dst, src_ps, AFT.Copy, bias=bias,
                                         scale=1.0)
                elif bias == 0.0:
                    nc.vector.tensor_copy(dst, src_ps)
                else:
                    nc.vector.tensor_scalar_add(dst, src_ps, bias)

            for p in range(planes):
                X = xyp.tile([128, 2048], F32, tag="X")
                Y = xyp.tile([128, 2048], F32, tag="Y")
                nch = C.get("dma_chunks", 1)
                for ch in range(nch):
                    kt_per = 4 // nch
                    sl = slice(ch * kt_per * 512, (ch + 1) * kt_per * 512)
                    rsl = slice(ch * kt_per * 128, (ch + 1) * kt_per * 128)
                    nc.sync.dma_start(
                        X[:, sl].rearrange("q (kt c) -> q kt c", kt=kt_per),
                        pred_d[p, rsl].rearrange("(kt q) c -> q kt c", q=128))
                    nc.sync.dma_start(
                        Y[:, sl].rearrange("q (kt c) -> q kt c", kt=kt_per),
                        targ_d[p, rsl].rearrange("(kt q) c -> q kt c", q=128))

                F1 = fldp.tile([128, 2048], BF16, tag="F1")
                F2 = fldp.tile([128, 2048], BF16, tag="F2")
                uF = fldp.tile([128, 2048], BF16, tag="uF")
                vF = fldp.tile([128, 2048], BF16, tag="vF")
                fch = C.get("field_chunks", 1)
                for ch in range(nch * fch):
                    w = 2048 // (nch * fch)
                    sl = slice(ch * w, (ch + 1) * w)
                    tt(C["f1"], F1[:, sl], X[:, sl], Y[:, sl], AOP.add)
                    tt(C["f2"], F2[:, sl], X[:, sl], Y[:, sl], AOP.subtract)
                    tt(C.get("ufvf", "dve"), uF[:, sl], F1[:, sl], F1[:, sl],
                       AOP.mult)
                    tt(C.get("ufvf", "dve"), vF[:, sl], F2[:, sl], F2[:, sl],
                       AOP.mult)

                TF1 = trp.tile([128, 2048], BF16, tag="TF1")
                TF2 = trp.tile([128, 2048], BF16, tag="TF2")
                TU = trp.tile([128, 2048], BF16, tag="TU")
                TW = trp.tile([128, 2048], BF16, tag="TW")

                dw = C["drain_w"]
                nblk = dw // 512
                for nm, T, srcs, bias in (
                    ("tf1", TF1, [(F1, 0)], 0.0),
                    ("tf2", TF2, [(F2, 0)], 0.0),
                    ("tu", TU, [(uF, 1), (vF, 1)], C2),
                    ("tw", TW, [(uF, 1), (vF, 2)], C2),
                ):
                    for mc0 in range(0, 4, nblk):
                        pf = ps1.tile([128, dw], F32, tag="p1")
                        for j in range(nblk):
                            conv_pass(pf[:, j * 512:(j + 1) * 512],
                                      srcs, mc0 + j)
                        drain(C[nm], T[:, mc0 * 512: mc0 * 512 + dw],
                              pf[:], bias)

                if C["acc"] == "pe":
                    SSACC = psaccp.tile([128, 512], F32, tag="ssacc")
                elif C["acc"] == "pool":
                    SA = pp.tile([128, 512], F32, tag="SA")
                    nc.gpsimd.memset(SA[:], 0.0)
                pair = {}

                hp = C.get("hp_offset", 0)
                for rc in range(4):
                    ssd = ps2a.tile([128, 1024], F32, tag="ssd")
                    pa2 = ps2b.tile([128, 512], F32, tag="a2")
                    pb2 = ps2b.tile([128, 512], F32, tag="b2")
                    conv_pass(ssd[:, 0:512], [(TF1, 0)], rc)
                    conv_pass(ssd[:, 512:1024], [(TF2, 0)], rc)
                    conv_pass(pa2, [(TW, 0)], rc, stop=False)
                    conv_pass(pb2, [(TU, 0)], rc, stop=False)

                    hpc = tc.high_priority(offset=hp) if hp else None
                    if hpc:
                        hpc.__enter__()
                    UV = pp.tile([128, 1024], BF16, tag="UV")
                    nc.scalar.activation(UV[:], ssd[:], AFT.Square, scale=INVR2)
                    u = UV[:, 0:512]
                    v = UV[:, 512:1024]

                    # A1 = u - v, B1 = u + v (C1 folded into the amr bias
                    # below; the TU/TW drain bias is C2 accordingly)
                    eng = C["a1b1"]
                    A1t = pp.tile([128, 512], BF16, tag="A1")
                    B1t = pp.tile([128, 512], BF16, tag="B1")
                    tt(eng, A1t[:], u, v, AOP.subtract)
                    tt(eng, B1t[:], u, v, AOP.add)
                    A1 = A1t[:]
                    B1 = B1t[:]
                    nc.tensor.matmul(pa2[:], negI, A1t[:],
                                     start=False, stop=True)
                    nc.tensor.matmul(pb2[:], negI, B1t[:],
                                     start=False, stop=True)

                    # m1 = (A1 + C1) * A2,  Dt = (B1 + C1) * B2
                    if rc % 2 == 0:
                        DT = pp.tile([128, 1024], BF16, tag="DT")
                        pair["DT"] = DT
                    else:
                        DT = pair["DT"]
                    h = (rc % 2) * 512
                    m1 = pp.tile([128, 512], BF16, tag="m1")
                    nc.vector.affine_mul_reduce(
                        m1[:], scr_acc[:, 0:1], A1, pa2[:], 1.0, C1)
                    pair[("m1", rc % 2)] = m1
                    nc.vector.affine_mul_reduce(
                        DT[:, h:h + 512], scr_acc[:, 1:2], B1, pb2[:],
                        1.0, C1)

                    if rc % 2 == 1:
                        RSQ = pp.tile([128, 1024], BF16, tag="RSQ")
                        nc.scalar.activation(RSQ[:], DT[:],
                                             AFT.Abs_reciprocal_sqrt, scale=1.0)
                        RR = pp.tile([128, 1024], BF16, tag="RR")
                        if C["rr"] == "act":
                            nc.scalar.activation(RR[:], RSQ[:], AFT.Square,
                                                 scale=1.0)
                        else:
                            tt(C["rr"], RR[:], RSQ[:], RSQ[:], AOP.mult)
                        for half in (0, 1):
                            rch = rc - 1 + half
                            RRh = RR[:, half * 512: half * 512 + 512]
                            m1h = pair[("m1", half)][:]
                            if C["acc"] == "amr":
                                scrap = pp.tile([128, 512], BF16, tag="scrap")
                                if C.get("final", "amr") == "ttr":
                                    nc.vector.tensor_tensor_reduce(
                                        out=scrap[:], in0=m1h, in1=RRh,
                                        scale=1.0, scalar=0.0, op0=AOP.mult,
                                        op1=AOP.add,
                                        accum_out=acc[:, p * 4 + rch:
                                                      p * 4 + rch + 1])
                                else:
                                    nc.vector.affine_mul_reduce(
                                        scrap[:],
                                        acc[:, p * 4 + rch: p * 4 + rch + 1],
                                        m1h, RRh, 1.0, 0.0)
                                continue
                            ss = pp.tile([128, 512], BF16, tag="ssim")
                            tt(C["ssim_eng"], ss[:], m1h, RRh, AOP.mult)
                            if C["acc"] == "pe":
                                nc.tensor.matmul(
                                    SSACC[:], posI, ss[:],
                                    start=(rch == 0), stop=(rch == 3))
                            else:
                                nc.gpsimd.tensor_tensor(
                                    SA[:], SA[:], ss[:], AOP.add)
                    if hpc:
                        hpc.__exit__(None, None, None)

                if C["acc"] == "pe":
                    nc.vector.tensor_reduce(
                        acc[:, p * 4: p * 4 + 1], SSACC[:],
                        mybir.AxisListType.X, AOP.add)
                elif C["acc"] == "pool":
                    nc.vector.tensor_reduce(
                        acc[:, p * 4: p * 4 + 1], SA[:],
                        mybir.AxisListType.X, AOP.add)

            nc.sync.dma_start(acc_d[:], acc[:])
    nc.compile()
    return nc


_CACHE = {}


def _get_nc():
    if "nc" not in _CACHE:
        _CACHE["nc"] = build_nc()
        _CACHE["consts"] = _consts_np()
    return _CACHE["nc"], _CACHE["consts"]


def kernel(pred, target, _trace=False):
    pred = np.ascontiguousarray(np.asarray(pred), dtype=np.float32)
    target = np.ascontiguousarray(np.asarray(target), dtype=np.float32)
    nc, consts = _get_nc()
    per = BATCH // N_CORES
    in_maps = []
    for i in range(N_CORES):
        in_maps.append({
            "pred": np.ascontiguousarray(
                pred[per * i: per * (i + 1)].reshape(PLANES, H, W)),
            "target": np.ascontiguousarray(
                target[per * i: per * (i + 1)].reshape(PLANES, H, W)),
            "consts": consts,
        })
    kw = {}
    if _trace:
        kw["trace"] = True
    res = run_bass_kernel_spmd(nc, in_maps, list(range(N_CORES)), **kw)
    total = 0.0
    for r in res.results:
        total += float(np.asarray(r["acc"]).astype(np.float64).sum())
    loss = 1.0 - total / float(BATCH * CH * H * W)
    out = np.float32(loss)
    if _trace:
        return out, res
    return out


# revision 3
# speedup vs baseline: 1.0023x; 1.0023x over previous
"""SSIM loss kernel v2 for Trainium2 (Bass/Tile), 8-core data parallel.

Math (per 512x512 plane, 11x11 gaussian window G, zero "same" padding):
  F1 = X+Y, F2 = X-Y, uF = F1^2/2, vF = F2^2/2
  s = G2(G1 F1), d = G2(G1 F2)        (separable blur via banded matmuls)
  U' = G2(G1 (uF+vF) + CC) = sg_x+sg_y + mu_x^2+mu_y^2 + CC  (interior)
  W' = G2(G1 (uF-vF) + CC) = 2 sg_xy + 2 mu_x mu_y + CC      (CC = C1+C2)
  u = s^2/2, v = d^2/2
  A1 = u-v+C1, B1 = u+v+C1, A2 = W'-A1, B2 = U'-B1
  ssim = (A1*A2)/(B1*B2);  loss = 1 - mean(ssim)

vs v1: bf16 matmuls (1 cyc/row vs 4), banded windows (542 vs 960 rows per
src-block), A2/B2 closed on the TensorEngine via negated-identity accumulate,
per-plane ssim summed in PSUM via identity accumulate, CC folded into pass1
copy bias, 1/Dt via Abs_reciprocal_sqrt, elementwise work spread across
DVE / Act / Pool (Pool is SBUF-only tensor_tensor/tensor_scalar/copy).
"""

import sys

for _p in ("/opt/trn_rl_repo",):
    if _p not in sys.path:
        sys.path.insert(0, _p)

import numpy as np
import ml_dtypes

import concourse.bass as bass
import concourse.bacc as bacc
import concourse.mybir as mybir
import concourse.tile as tile
from concourse.bass_utils import run_bass_kernel_spmd

F32 = mybir.dt.float32
BF16 = mybir.dt.bfloat16
AOP = mybir.AluOpType
AFT = mybir.ActivationFunctionType

N_CORES = 8
BATCH = 16
CH = 3
H = W = 512
PLANES = (BATCH // N_CORES) * CH  # 6 planes per core
WIN_SIZE = 11
SIGMA = 1.5
HALF = WIN_SIZE // 2
C1 = 0.01 ** 2
C2 = 0.03 ** 2
CC = C1 + C2
INVR2 = float(np.float32(1.0) / np.sqrt(np.float32(2.0)))

WIN = [(0, 133), (123, 138), (251, 138), (379, 133)]
OFF = [0, 133, 271, 409]
CATW = 542
CONSTW = 3 * CATW + 256  # 3 band copies + negI + posI

# default engine/strategy choices (overridable for the sim-driven search)
CFG = dict(
    tf1="act",      # TF1 drain: dve | act
    tf2="act",      # TF2 drain: dve | act
    tu="act",       # TU drain (+CC bias): act | dve
    tw="act",       # TW drain (+CC bias): act | dve
    a1b1="dve",     # A1/B1 combine: dve | pool
    f1="dve",       # F1: dve | pool
    f2="pool",      # F2: dve | pool
    rr="dve",       # RR = RSQ^2: pool | dve
    ssim_eng="dve", # final ssim product: dve | pool
    drain_w=1024,   # pass1 drain width: 512 | 1024 | 2048
    acc="amr",      # ssim accumulate: pe (psum identity) | pool (sbuf adds)
                    #                  | amr (DVE fused product+reduce)
    ps1_bufs=None,  # pass1 psum bufs (None = auto)
    ps2b_bufs=1,    # a2/b2 psum bufs
    dma_chunks=2,   # input DMA split for earlier pipeline start
    dma_chunks0=4,  # finer split on plane 0 (pipeline fill)
)


def _gauss1d():
    coords = np.arange(WIN_SIZE, dtype=np.float32) - HALF
    g = np.exp(-(coords ** 2) / np.float32(2.0 * SIGMA ** 2)).astype(np.float32)
    g = g / g.sum(dtype=np.float32)
    return g.astype(np.float32)


def _band_np():
    g = _gauss1d()
    A = np.zeros((H, H), dtype=np.float32)
    for i in range(H):
        lo = max(0, i - HALF)
        hi = min(H, i + HALF + 1)
        for j in range(lo, hi):
            A[i, j] = g[j - i + HALF]
    segs = []
    for kt in range(4):
        ns, w = WIN[kt]
        segs.append(A[ns:ns + w, kt * 128:(kt + 1) * 128].T.copy())
    cat = np.concatenate(segs, axis=1)
    assert cat.shape == (128, CATW)
    return cat


def _consts_np():
    """[128, 3*CATW + 256] bf16: [BM | 0.5*BM | -0.5*BM | -I | I]."""
    cat = _band_np()
    eye = np.eye(128, dtype=np.float32)
    full = np.concatenate([cat, 0.5 * cat, -0.5 * cat, -eye, eye], axis=1)
    return full.astype(ml_dtypes.bfloat16)


def build_nc(planes=PLANES, cfg=None):
    C = dict(CFG)
    if cfg:
        C.update(cfg)

    ps1_bufs = C["ps1_bufs"]
    if ps1_bufs is None:
        ps1_bufs = 2 if C["drain_w"] <= 1024 else 1

    nc = bacc.Bacc(None)
    pred_d = nc.declare_dram_parameter("pred", [planes, H, W], F32, isOutput=False)
    targ_d = nc.declare_dram_parameter("target", [planes, H, W], F32, isOutput=False)
    cst_d = nc.declare_dram_parameter("consts", [128, CONSTW], BF16, isOutput=False)
    acc_d = nc.declare_dram_parameter("acc", [128, planes * 4], F32, isOutput=True)

    def tt(eng, out, a, b, op):
        getattr(nc, {"dve": "vector", "pool": "gpsimd"}[eng]).tensor_tensor(
            out, a, b, op)

    with tile.TileContext(nc) as tc:
        with (
            tc.tile_pool(name="const", bufs=1) as constp,
            tc.tile_pool(name="xy", bufs=2) as xyp,
            tc.tile_pool(name="fields", bufs=C.get("fld_bufs", 2)) as fldp,
            tc.tile_pool(name="transposed", bufs=2) as trp,
            tc.tile_pool(name="post", bufs=2) as pp,
            tc.tile_pool(name="accp", bufs=1) as accp,
            tc.tile_pool(name="ps1", bufs=ps1_bufs, space="PSUM") as ps1,
            tc.tile_pool(name="ps2a", bufs=1, space="PSUM") as ps2a,
            tc.tile_pool(name="ps2b", bufs=C["ps2b_bufs"], space="PSUM") as ps2b,
            tc.tile_pool(name="psacc", bufs=1, space="PSUM") as psaccp,
        ):
            CST = constp.tile([128, CONSTW], BF16)
            nc.sync.dma_start(CST[:], cst_d[:])
            negI = CST[:, 3 * CATW: 3 * CATW + 128]
            posI = CST[:, 3 * CATW + 128: 3 * CATW + 256]
            acc = accp.tile([128, planes * 4], F32)
            nc.vector.memset(acc[:], 0.0)
            scr_acc = accp.tile([128, 2], F32)
            nc.vector.memset(scr_acc[:], 0.0)

            def conv_pass(dst, srcs, blk, stop=True):
                mms = []
                for (T, seg) in srcs:
                    for kt in range(4):
                        lhsT = T[:, kt * 512 + blk * 128: kt * 512 + (blk + 1) * 128]
                        ns, w = WIN[kt]
                        off = seg * CATW + OFF[kt]
                        mms.append((dst[:, ns:ns + w], lhsT, CST[:, off:off + w]))
                n_mm = len(mms)
                for i, (o, l, r) in enumerate(mms):
                    nc.tensor.matmul(o, l, r, start=(i == 0),
                                     stop=(stop and i == n_mm - 1))

            def drain(eng, dst, src_ps, bias):
                if eng == "act":
                    nc.scalar.activation(dst, src_ps, AFT.Copy, bias=bias,
                                         scale=1.0)
                elif bias == 0.0:
                    nc.vector.tensor_copy(dst, src_ps)
                else:
                    nc.vector.tensor_scalar_add(dst, src_ps, bias)

            for p in range(planes):
                X = xyp.tile([128, 2048], F32, tag="X")
                Y = xyp.tile([128, 2048], F32, tag="Y")
                nch = C.get("dma_chunks", 1)
                if p == 0:
                    nch = C.get("dma_chunks0", nch)
                for ch in range(nch):
                    kt_per = 4 // nch
                    sl = slice(ch * kt_per * 512, (ch + 1) * kt_per * 512)
                    rsl = slice(ch * kt_per * 128, (ch + 1) * kt_per * 128)
                    nc.sync.dma_start(
                        X[:, sl].rearrange("q (kt c) -> q kt c", kt=kt_per),
                        pred_d[p, rsl].rearrange("(kt q) c -> q kt c", q=128))
                    nc.sync.dma_start(
                        Y[:, sl].rearrange("q (kt c) -> q kt c", kt=kt_per),
                        targ_d[p, rsl].rearrange("(kt q) c -> q kt c", q=128))

                F1 = fldp.tile([128, 2048], BF16, tag="F1")
                F2 = fldp.tile([128, 2048], BF16, tag="F2")
                uF = fldp.tile([128, 2048], BF16, tag="uF")
                vF = fldp.tile([128, 2048], BF16, tag="vF")
                fch = C.get("field_chunks", 1)
                for ch in range(nch * fch):
                    w = 2048 // (nch * fch)
                    sl = slice(ch * w, (ch + 1) * w)
                    tt(C["f1"], F1[:, sl], X[:, sl], Y[:, sl], AOP.add)
                    tt(C["f2"], F2[:, sl], X[:, sl], Y[:, sl], AOP.subtract)
                    tt(C.get("ufvf", "dve"), uF[:, sl], F1[:, sl], F1[:, sl],
                       AOP.mult)
                    tt(C.get("ufvf", "dve"), vF[:, sl], F2[:, sl], F2[:, sl],
                       AOP.mult)

                TF1 = trp.tile([128, 2048], BF16, tag="TF1")
                TF2 = trp.tile([128, 2048], BF16, tag="TF2")
                TU = trp.tile([128, 2048], BF16, tag="TU")
                TW = trp.tile([128, 2048], BF16, tag="TW")

                dw = C["drain_w"]
                nblk = dw // 512
                for nm, T, srcs, bias in (
                    ("tf1", TF1, [(F1, 0)], 0.0),
                    ("tf2", TF2, [(F2, 0)], 0.0),
                    ("tu", TU, [(uF, 1), (vF, 1)], C2),
                    ("tw", TW, [(uF, 1), (vF, 2)], C2),
                ):
                    for mc0 in range(0, 4, nblk):
                        pf = ps1.tile([128, dw], F32, tag="p1")
                        for j in range(nblk):
                            conv_pass(pf[:, j * 512:(j + 1) * 512],
                                      srcs, mc0 + j)
                        drain(C[nm], T[:, mc0 * 512: mc0 * 512 + dw],
                              pf[:], bias)

                if C["acc"] == "pe":
                    SSACC = psaccp.tile([128, 512], F32, tag="ssacc")
                elif C["acc"] == "pool":
                    SA = pp.tile([128, 512], F32, tag="SA")
                    nc.gpsimd.memset(SA[:], 0.0)
                pair = {}

                hp = C.get("hp_offset", 0)
                for rc in range(4):
                    ssd = ps2a.tile([128, 1024], F32, tag="ssd")
                    pa2 = ps2b.tile([128, 512], F32, tag="a2")
                    pb2 = ps2b.tile([128, 512], F32, tag="b2")
                    conv_pass(ssd[:, 0:512], [(TF1, 0)], rc)
                    conv_pass(ssd[:, 512:1024], [(TF2, 0)], rc)
                    conv_pass(pa2, [(TW, 0)], rc, stop=False)
                    conv_pass(pb2, [(TU, 0)], rc, stop=False)

                    hpc = tc.high_priority(offset=hp) if hp else None
                    if hpc:
                        hpc.__enter__()
                    UV = pp.tile([128, 1024], BF16, tag="UV")
                    nc.scalar.activation(UV[:], ssd[:], AFT.Square, scale=INVR2)
                    u = UV[:, 0:512]
                    v = UV[:, 512:1024]

                    # A1 = u - v, B1 = u + v (C1 folded into the amr bias
                    # below; the TU/TW drain bias is C2 accordingly)
                    eng = C["a1b1"]
                    A1t = pp.tile([128, 512], BF16, tag="A1")
                    B1t = pp.tile([128, 512], BF16, tag="B1")
                    tt(eng, A1t[:], u, v, AOP.subtract)
                    tt(eng, B1t[:], u, v, AOP.add)
                    A1 = A1t[:]
                    B1 = B1t[:]
                    nc.tensor.matmul(pa2[:], negI, A1t[:],
                                     start=False, stop=True)
                    nc.tensor.matmul(pb2[:], negI, B1t[:],
                                     start=False, stop=True)

                    # m1 = (A1 + C1) * A2,  Dt = (B1 + C1) * B2
                    if rc % 2 == 0:
                        DT = pp.tile([128, 1024], BF16, tag="DT")
                        pair["DT"] = DT
                    else:
                        DT = pair["DT"]
                    h = (rc % 2) * 512
                    m1 = pp.tile([128, 512], BF16, tag="m1")
                    nc.vector.affine_mul_reduce(
                        m1[:], scr_acc[:, 0:1], A1, pa2[:], 1.0, C1)
                    pair[("m1", rc % 2)] = m1
                    nc.vector.affine_mul_reduce(
                        DT[:, h:h + 512], scr_acc[:, 1:2], B1, pb2[:],
                        1.0, C1)

                    if rc % 2 == 1:
                        RSQ = pp.tile([128, 1024], BF16, tag="RSQ")
                        nc.scalar.activation(RSQ[:], DT[:],
                                             AFT.Abs_reciprocal_sqrt, scale=1.0)
                        RR = pp.tile([128, 1024], BF16, tag="RR")
                        if C["rr"] == "act":
                            nc.scalar.activation(RR[:], RSQ[:], AFT.Square,
                                                 scale=1.0)
                        else:
                            tt(C["rr"], RR[:], RSQ[:], RSQ[:], AOP.mult)
                        for half in (0, 1):
                            rch = rc - 1 + half
                            RRh = RR[:, half * 512: half * 512 + 512]
                            m1h = pair[("m1", half)][:]
                            if C["acc"] == "amr":
                                scrap = pp.tile([128, 512], BF16, tag="scrap")
                                if C.get("final", "amr") == "ttr":
                                    nc.vector.tensor_tensor_reduce(
                                        out=scrap[:], in0=m1h, in1=RRh,
                                        scale=1.0, scalar=0.0, op0=AOP.mult,
                                        op1=AOP.add,
                                        accum_out=acc[:, p * 4 + rch:
                                                      p * 4 + rch + 1])
                                else:
                                    nc.vector.affine_mul_reduce(
                                        scrap[:],
                                        acc[:, p * 4 + rch: p * 4 + rch + 1],
                                        m1h, RRh, 1.0, 0.0)
                                continue
                            ss = pp.tile([128, 512], BF16, tag="ssim")
                            tt(C["ssim_eng"], ss[:], m1h, RRh, AOP.mult)
                            if C["acc"] == "pe":
                                nc.tensor.matmul(
                                    SSACC[:], posI, ss[:],
                                    start=(rch == 0), stop=(rch == 3))
                            else:
                                nc.gpsimd.tensor_tensor(
                                    SA[:], SA[:], ss[:], AOP.add)
                    if hpc:
                        hpc.__exit__(None, None, None)

                if C["acc"] == "pe":
                    nc.vector.tensor_reduce(
                        acc[:, p * 4: p * 4 + 1], SSACC[:],
                        mybir.AxisListType.X, AOP.add)
                elif C["acc"] == "pool":
                    nc.vector.tensor_reduce(
                        acc[:, p * 4: p * 4 + 1], SA[:],
                        mybir.AxisListType.X, AOP.add)

            nc.sync.dma_start(acc_d[:], acc[:])
    nc.compile()
    return nc


_CACHE = {}


def _get_nc():
    if "nc" not in _CACHE:
        _CACHE["nc"] = build_nc()
        _CACHE["consts"] = _consts_np()
    return _CACHE["nc"], _CACHE["consts"]


def kernel(pred, target, _trace=False):
    pred = np.ascontiguousarray(np.asarray(pred), dtype=np.float32)
    target = np.ascontiguousarray(np.asarray(target), dtype=np.float32)
    nc, consts = _get_nc()
    per = BATCH // N_CORES
    in_maps = []
    for i in range(N_CORES):
        in_maps.append({
            "pred": np.ascontiguousarray(
                pred[per * i: per * (i + 1)].reshape(PLANES, H, W)),
            "target": np.ascontiguousarray(
                target[per * i: per * (i + 1)].reshape(PLANES, H, W)),
            "consts": consts,
        })
    kw = {}
    if _trace:
        kw["trace"] = True
    res = run_bass_kernel_spmd(nc, in_maps, list(range(N_CORES)), **kw)
    total = 0.0
    for r in res.results:
        total += float(np.asarray(r["acc"]).astype(np.float64).sum())
    loss = 1.0 - total / float(BATCH * CH * H * W)
    out = np.float32(loss)
    if _trace:
        return out, res
    return out


# revision 4
# speedup vs baseline: 1.0186x; 1.0162x over previous
"""SSIM loss kernel v2 for Trainium2 (Bass/Tile), 8-core data parallel.

Math (per 512x512 plane, 11x11 gaussian window G, zero "same" padding):
  F1 = X+Y, F2 = X-Y, uF = F1^2/2, vF = F2^2/2
  s = G2(G1 F1), d = G2(G1 F2)        (separable blur via banded matmuls)
  U' = G2(G1 (uF+vF) + CC) = sg_x+sg_y + mu_x^2+mu_y^2 + CC  (interior)
  W' = G2(G1 (uF-vF) + CC) = 2 sg_xy + 2 mu_x mu_y + CC      (CC = C1+C2)
  u = s^2/2, v = d^2/2
  A1 = u-v+C1, B1 = u+v+C1, A2 = W'-A1, B2 = U'-B1
  ssim = (A1*A2)/(B1*B2);  loss = 1 - mean(ssim)

vs v1: bf16 matmuls (1 cyc/row vs 4), banded windows (542 vs 960 rows per
src-block), A2/B2 closed on the TensorEngine via negated-identity accumulate,
per-plane ssim summed in PSUM via identity accumulate, CC folded into pass1
copy bias, 1/Dt via Abs_reciprocal_sqrt, elementwise work spread across
DVE / Act / Pool (Pool is SBUF-only tensor_tensor/tensor_scalar/copy).
"""

import sys

for _p in ("/opt/trn_rl_repo",):
    if _p not in sys.path:
        sys.path.insert(0, _p)

import numpy as np
import ml_dtypes

import concourse.bass as bass
import concourse.bacc as bacc
import concourse.mybir as mybir
import concourse.tile as tile
from concourse.bass_utils import run_bass_kernel_spmd

F32 = mybir.dt.float32
BF16 = mybir.dt.bfloat16
AOP = mybir.AluOpType
AFT = mybir.ActivationFunctionType

N_CORES = 8
BATCH = 16
CH = 3
H = W = 512
PLANES = (BATCH // N_CORES) * CH  # 6 planes per core
WIN_SIZE = 11
SIGMA = 1.5
HALF = WIN_SIZE // 2
C1 = 0.01 ** 2
C2 = 0.03 ** 2
CC = C1 + C2
INVR2 = float(np.float32(1.0) / np.sqrt(np.float32(2.0)))

WIN = [(0, 133), (123, 138), (251, 138), (379, 133)]
OFF = [0, 133, 271, 409]
CATW = 542
CONSTW = 3 * CATW + 256  # 3 band copies + negI + posI

# default engine/strategy choices (overridable for the sim-driven search)
CFG = dict(
    tf1="act",      # TF1 drain: dve | act
    tf2="act",      # TF2 drain: dve | act
    tu="act",       # TU drain (+CC bias): act | dve
    tw="act",       # TW drain (+CC bias): act | dve
    a1b1="dve",     # A1/B1 combine: dve | pool
    f1="dve",       # F1: dve | pool
    f2="dve",       # F2: dve | pool
    ufvf="pool",    # uF/vF squares: dve | pool
    rr="dve",       # RR = RSQ^2: pool | dve
    ssim_eng="dve", # final ssim product: dve | pool
    drain_w=1024,   # pass1 drain width: 512 | 1024 | 2048
    acc="amr",      # ssim accumulate: pe (psum identity) | pool (sbuf adds)
                    #                  | amr (DVE fused product+reduce)
    ps1_bufs=None,  # pass1 psum bufs (None = auto)
    ps2b_bufs=1,    # a2/b2 psum bufs
    dma_chunks=2,   # input DMA split for earlier pipeline start
    dma_chunks0=4,  # finer split on plane 0 (pipeline fill)
)


def _gauss1d():
    coords = np.arange(WIN_SIZE, dtype=np.float32) - HALF
    g = np.exp(-(coords ** 2) / np.float32(2.0 * SIGMA ** 2)).astype(np.float32)
    g = g / g.sum(dtype=np.float32)
    return g.astype(np.float32)


def _band_np():
    g = _gauss1d()
    A = np.zeros((H, H), dtype=np.float32)
    for i in range(H):
        lo = max(0, i - HALF)
        hi = min(H, i + HALF + 1)
        for j in range(lo, hi):
            A[i, j] = g[j - i + HALF]
    segs = []
    for kt in range(4):
        ns, w = WIN[kt]
        segs.append(A[ns:ns + w, kt * 128:(kt + 1) * 128].T.copy())
    cat = np.concatenate(segs, axis=1)
    assert cat.shape == (128, CATW)
    return cat


def _consts_np():
    """[128, 3*CATW + 256] bf16: [BM | 0.5*BM | -0.5*BM | -I | I]."""
    cat = _band_np()
    eye = np.eye(128, dtype=np.float32)
    full = np.concatenate([cat, 0.5 * cat, -0.5 * cat, -eye, eye], axis=1)
    return full.astype(ml_dtypes.bfloat16)


def build_nc(planes=PLANES, cfg=None):
    C = dict(CFG)
    if cfg:
        C.update(cfg)

    ps1_bufs = C["ps1_bufs"]
    if ps1_bufs is None:
        ps1_bufs = 2 if C["drain_w"] <= 1024 else 1

    nc = bacc.Bacc(None)
    pred_d = nc.declare_dram_parameter("pred", [planes, H, W], F32, isOutput=False)
    targ_d = nc.declare_dram_parameter("target", [planes, H, W], F32, isOutput=False)
    cst_d = nc.declare_dram_parameter("consts", [128, CONSTW], BF16, isOutput=False)
    acc_d = nc.declare_dram_parameter("acc", [128, planes * 4], F32, isOutput=True)

    def tt(eng, out, a, b, op):
        getattr(nc, {"dve": "vector", "pool": "gpsimd"}[eng]).tensor_tensor(
            out, a, b, op)

    with tile.TileContext(nc) as tc:
        with (
            tc.tile_pool(name="const", bufs=1) as constp,
            tc.tile_pool(name="xy", bufs=2) as xyp,
            tc.tile_pool(name="fields", bufs=C.get("fld_bufs", 2)) as fldp,
            tc.tile_pool(name="transposed", bufs=2) as trp,
            tc.tile_pool(name="post", bufs=2) as pp,
            tc.tile_pool(name="accp", bufs=1) as accp,
            tc.tile_pool(name="ps1", bufs=ps1_bufs, space="PSUM") as ps1,
            tc.tile_pool(name="ps2a", bufs=1, space="PSUM") as ps2a,
            tc.tile_pool(name="ps2b", bufs=C["ps2b_bufs"], space="PSUM") as ps2b,
            tc.tile_pool(name="psacc", bufs=1, space="PSUM") as psaccp,
        ):
            CST = constp.tile([128, CONSTW], BF16)
            nc.sync.dma_start(CST[:], cst_d[:])
            negI = CST[:, 3 * CATW: 3 * CATW + 128]
            posI = CST[:, 3 * CATW + 128: 3 * CATW + 256]
            acc = accp.tile([128, planes * 4], F32)
            nc.vector.memset(acc[:], 0.0)
            scr_acc = accp.tile([128, 2], F32)
            nc.vector.memset(scr_acc[:], 0.0)

            def conv_pass(dst, srcs, blk, stop=True):
                mms = []
                for (T, seg) in srcs:
                    for kt in range(4):
                        lhsT = T[:, kt * 512 + blk * 128: kt * 512 + (blk + 1) * 128]
                        ns, w = WIN[kt]
                        off = seg * CATW + OFF[kt]
                        mms.append((dst[:, ns:ns + w], lhsT, CST[:, off:off + w]))
                n_mm = len(mms)
                for i, (o, l, r) in enumerate(mms):
                    nc.tensor.matmul(o, l, r, start=(i == 0),
                                     stop=(stop and i == n_mm - 1))

            def drain(eng, dst, src_ps, bias):
                if eng == "act":
                    nc.scalar.activation(dst, src_ps, AFT.Copy, bias=bias,
                                         scale=1.0)
                elif bias == 0.0:
                    nc.vector.tensor_copy(dst, src_ps)
                else:
                    nc.vector.tensor_scalar_add(dst, src_ps, bias)

            for p in range(planes):
                X = xyp.tile([128, 2048], F32, tag="X")
                Y = xyp.tile([128, 2048], F32, tag="Y")
                nch = C.get("dma_chunks", 1)
                if p == 0:
                    nch = C.get("dma_chunks0", nch)
                for ch in range(nch):
                    kt_per = 4 // nch
                    sl = slice(ch * kt_per * 512, (ch + 1) * kt_per * 512)
                    rsl = slice(ch * kt_per * 128, (ch + 1) * kt_per * 128)
                    nc.sync.dma_start(
                        X[:, sl].rearrange("q (kt c) -> q kt c", kt=kt_per),
                        pred_d[p, rsl].rearrange("(kt q) c -> q kt c", q=128))
                    nc.sync.dma_start(
                        Y[:, sl].rearrange("q (kt c) -> q kt c", kt=kt_per),
                        targ_d[p, rsl].rearrange("(kt q) c -> q kt c", q=128))

                F1 = fldp.tile([128, 2048], BF16, tag="F1")
                F2 = fldp.tile([128, 2048], BF16, tag="F2")
                uF = fldp.tile([128, 2048], BF16, tag="uF")
                vF = fldp.tile([128, 2048], BF16, tag="vF")
                fch = C.get("field_chunks", 1)
                for ch in range(nch * fch):
                    w = 2048 // (nch * fch)
                    sl = slice(ch * w, (ch + 1) * w)
                    tt(C["f1"], F1[:, sl], X[:, sl], Y[:, sl], AOP.add)
                    tt(C["f2"], F2[:, sl], X[:, sl], Y[:, sl], AOP.subtract)
                    tt(C.get("ufvf", "dve"), uF[:, sl], F1[:, sl], F1[:, sl],
                       AOP.mult)
                    tt(C.get("ufvf", "dve"), vF[:, sl], F2[:, sl], F2[:, sl],
                       AOP.mult)

                TF1 = trp.tile([128, 2048], BF16, tag="TF1")
                TF2 = trp.tile([128, 2048], BF16, tag="TF2")
                TU = trp.tile([128, 2048], BF16, tag="TU")
                TW = trp.tile([128, 2048], BF16, tag="TW")

                dw = C["drain_w"]
                nblk = dw // 512
                for nm, T, srcs, bias in (
                    ("tf1", TF1, [(F1, 0)], 0.0),
                    ("tf2", TF2, [(F2, 0)], 0.0),
                    ("tu", TU, [(uF, 1), (vF, 1)], C2),
                    ("tw", TW, [(uF, 1), (vF, 2)], C2),
                ):
                    for mc0 in range(0, 4, nblk):
                        pf = ps1.tile([128, dw], F32, tag="p1")
                        for j in range(nblk):
                            conv_pass(pf[:, j * 512:(j + 1) * 512],
                                      srcs, mc0 + j)
                        drain(C[nm], T[:, mc0 * 512: mc0 * 512 + dw],
                              pf[:], bias)

                if C["acc"] == "pe":
                    SSACC = psaccp.tile([128, 512], F32, tag="ssacc")
                elif C["acc"] == "pool":
                    SA = pp.tile([128, 512], F32, tag="SA")
                    nc.gpsimd.memset(SA[:], 0.0)
                pair = {}

                hp = C.get("hp_offset", 0)
                for rc in range(4):
                    ssd = ps2a.tile([128, 1024], F32, tag="ssd")
                    pa2 = ps2b.tile([128, 512], F32, tag="a2")
                    pb2 = ps2b.tile([128, 512], F32, tag="b2")
                    conv_pass(ssd[:, 0:512], [(TF1, 0)], rc)
                    conv_pass(ssd[:, 512:1024], [(TF2, 0)], rc)
                    conv_pass(pa2, [(TW, 0)], rc, stop=False)
                    conv_pass(pb2, [(TU, 0)], rc, stop=False)

                    hpc = tc.high_priority(offset=hp) if hp else None
                    if hpc:
                        hpc.__enter__()
                    UV = pp.tile([128, 1024], BF16, tag="UV")
                    nc.scalar.activation(UV[:], ssd[:], AFT.Square, scale=INVR2)
                    u = UV[:, 0:512]
                    v = UV[:, 512:1024]

                    # A1 = u - v, B1 = u + v (C1 folded into the amr bias
                    # below; the TU/TW drain bias is C2 accordingly)
                    eng = C["a1b1"]
                    A1t = pp.tile([128, 512], BF16, tag="A1")
                    B1t = pp.tile([128, 512], BF16, tag="B1")
                    tt(eng, A1t[:], u, v, AOP.subtract)
                    tt(eng, B1t[:], u, v, AOP.add)
                    A1 = A1t[:]
                    B1 = B1t[:]
                    nc.tensor.matmul(pa2[:], negI, A1t[:],
                                     start=False, stop=True)
                    nc.tensor.matmul(pb2[:], negI, B1t[:],
                                     start=False, stop=True)

                    # m1 = (A1 + C1) * A2,  Dt = (B1 + C1) * B2
                    if rc % 2 == 0:
                        DT = pp.tile([128, 1024], BF16, tag="DT")
                        pair["DT"] = DT
                    else:
                        DT = pair["DT"]
                    h = (rc % 2) * 512
                    m1 = pp.tile([128, 512], BF16, tag="m1")
                    nc.vector.affine_mul_reduce(
                        m1[:], scr_acc[:, 0:1], A1, pa2[:], 1.0, C1)
                    pair[("m1", rc % 2)] = m1
                    nc.vector.affine_mul_reduce(
                        DT[:, h:h + 512], scr_acc[:, 1:2], B1, pb2[:],
                        1.0, C1)

                    if rc % 2 == 1:
                        RSQ = pp.tile([128, 1024], BF16, tag="RSQ")
                        nc.scalar.activation(RSQ[:], DT[:],
                                             AFT.Abs_reciprocal_sqrt, scale=1.0)
                        RR = pp.tile([128, 1024], BF16, tag="RR")
                        if C["rr"] == "act":
                            nc.scalar.activation(RR[:], RSQ[:], AFT.Square,
                                                 scale=1.0)
                        else:
                            tt(C["rr"], RR[:], RSQ[:], RSQ[:], AOP.mult)
                        for half in (0, 1):
                            rch = rc - 1 + half
                            RRh = RR[:, half * 512: half * 512 + 512]
                            m1h = pair[("m1", half)][:]
                            if C["acc"] == "amr":
                                scrap = pp.tile([128, 512], BF16, tag="scrap")
                                if C.get("final", "amr") == "ttr":
                                    nc.vector.tensor_tensor_reduce(
                                        out=scrap[:], in0=m1h, in1=RRh,
                                        scale=1.0, scalar=0.0, op0=AOP.mult,
                                        op1=AOP.add,
                                        accum_out=acc[:, p * 4 + rch:
                                                      p * 4 + rch + 1])
                                else:
                                    nc.vector.affine_mul_reduce(
                                        scrap[:],
                                        acc[:, p * 4 + rch: p * 4 + rch + 1],
                                        m1h, RRh, 1.0, 0.0)
                                continue
                            ss = pp.tile([128, 512], BF16, tag="ssim")
                            tt(C["ssim_eng"], ss[:], m1h, RRh, AOP.mult)
                            if C["acc"] == "pe":
                                nc.tensor.matmul(
                                    SSACC[:], posI, ss[:],
                                    start=(rch == 0), stop=(rch == 3))
                            else:
                                nc.gpsimd.tensor_tensor(
                                    SA[:], SA[:], ss[:], AOP.add)
                    if hpc:
                        hpc.__exit__(None, None, None)

                if C["acc"] == "pe":
                    nc.vector.tensor_reduce(
                        acc[:, p * 4: p * 4 + 1], SSACC[:],
                        mybir.AxisListType.X, AOP.add)
                elif C["acc"] == "pool":
                    nc.vector.tensor_reduce(
                        acc[:, p * 4: p * 4 + 1], SA[:],
                        mybir.AxisListType.X, AOP.add)

            nc.sync.dma_start(acc_d[:], acc[:])
    nc.compile()
    return nc


_CACHE = {}


def _get_nc():
    if "nc" not in _CACHE:
        _CACHE["nc"] = build_nc()
        _CACHE["consts"] = _consts_np()
    return _CACHE["nc"], _CACHE["consts"]


def kernel(pred, target, _trace=False):
    pred = np.ascontiguousarray(np.asarray(pred), dtype=np.float32)
    target = np.ascontiguousarray(np.asarray(target), dtype=np.float32)
    nc, consts = _get_nc()
    per = BATCH // N_CORES
    in_maps = []
    for i in range(N_CORES):
        in_maps.append({
            "pred": np.ascontiguousarray(
                pred[per * i: per * (i + 1)].reshape(PLANES, H, W)),
            "target": np.ascontiguousarray(
                target[per * i: per * (i + 1)].reshape(PLANES, H, W)),
            "consts": consts,
        })
    kw = {}
    if _trace:
        kw["trace"] = True
    res = run_bass_kernel_spmd(nc, in_maps, list(range(N_CORES)), **kw)
    total = 0.0
    for r in res.results:
        total += float(np.asarray(r["acc"]).astype(np.float64).sum())
    loss = 1.0 - total / float(BATCH * CH * H * W)
    out = np.float32(loss)
    if _trace:
        return out, res
    return out
